# revision 6
# baseline (speedup 1.0000x reference)
"""Trainium2 Bass kernel for nn_Confidence_Loss_2 (grid-sample-nearest confidence loss).

Strategy: pure data parallel — 2 batch samples per NeuronCore across 8 cores.
Per core:
  - DVE computes nearest-neighbor sample indices (scale/clamp/round-half-even
    via the +2^23 trick) into a flat int32 index tile.
  - GPSIMD SWDGE indirect DMA gathers target[idx] from DRAM (the only
    per-element gather mechanism with acceptable throughput).
  - ACT computes log(f+eps) / log(1-f+eps) with fused per-partition
    accumulation; DVE builds the equality mask and the masked correction
    term, also with fused accumulation.
  - Host sums the tiny per-core [128, 8] partial tensors.
"""

import os

import numpy as np

import concourse.bacc as bacc
import concourse.mybir as mybir
import concourse.tile as tile
from concourse.bass import IndirectOffsetOnAxis
from concourse.bass_utils import run_bass_kernel_spmd

B, H, W = 16, 512, 1024
NCORES = 8
SPC = B // NCORES          # samples per core
P = 128
NPIX = H * W               # 524288
COLS = NPIX // P           # 4096
CHUNK = 2048               # free-dim chunk (half a sample)
NCHUNK = COLS // CHUNK     # chunks per sample
EPS = 1e-7
RC = float(1 << 23)        # round-to-nearest-even bias constant

F32 = mybir.dt.float32
I32 = mybir.dt.int32
Alu = mybir.AluOpType
Act = mybir.ActivationFunctionType

# number of indirect-gather splits per chunk (finer grain = better overlap
# of SWDGE descriptor generation with SDMA drain)
GSPLIT = int(os.environ.get("CONF_GSPLIT", "4"))


def build():
    nc = bacc.Bacc("TRN2", target_bir_lowering=False, debug=False)
    off_d = nc.dram_tensor("offset", [SPC, 2, H, W], F32, kind="ExternalInput")
    f_d = nc.dram_tensor("f", [SPC, H, W], F32, kind="ExternalInput")
    t_d = nc.dram_tensor("target", [SPC, H, W], I32, kind="ExternalInput")
    nacc = 2 * SPC * NCHUNK
    out_d = nc.dram_tensor("out", [P, nacc], F32, kind="ExternalOutput")

    # [SPC, 2, 128, 4096]: partition p holds image rows [4p, 4p+4)
    off_v = off_d.ap().rearrange("s c (p x) w -> s c p (x w)", p=P)
    f_v = f_d.ap().rearrange("s (p x) w -> s p (x w)", p=P)
    t_v = t_d.ap().rearrange("s (p x) w -> s p (x w)", p=P)
    tflat = t_d.ap().rearrange("s h w -> (s h w)").unsqueeze(-1)  # table, offset 0

    with tile.TileContext(nc) as tc:
        with (
            tc.tile_pool(name="persist", bufs=1) as pp,
            tc.tile_pool(name="work", bufs=2) as wp,
        ):
            # ---- one-time base coordinate tiles ----
            # chunk element (p, a*W + w) -> image pixel (h = 4p + 2*ch + a, w)
            base_x = pp.tile([P, CHUNK], F32, tag="base_x")
            base_ys = []
            nc.gpsimd.iota(
                base_x[:].rearrange("p (a w) -> p a w", w=W),
                pattern=[[0, CHUNK // W], [1, W]],
                base=0,
                channel_multiplier=0,
                allow_small_or_imprecise_dtypes=True,
            )
            # ix = off_x*W/2 + (w*W/(W-1) - 0.5)
            nc.vector.tensor_scalar(
                base_x[:], base_x[:], float(W) / (W - 1), 0.5, Alu.mult, Alu.subtract
            )
            for ch in range(NCHUNK):
                by = pp.tile([P, CHUNK], F32, tag=f"base_y{ch}")
                nc.gpsimd.iota(
                    by[:].rearrange("p (a w) -> p a w", w=W),
                    pattern=[[1, CHUNK // W], [0, W]],
                    base=(CHUNK // W) * ch,
                    channel_multiplier=COLS // W,
                    allow_small_or_imprecise_dtypes=True,
                )
                nc.vector.tensor_scalar(
                    by[:], by[:], float(H) / (H - 1), 0.5, Alu.mult, Alu.subtract
                )
                base_ys.append(by)
            racc = pp.tile([P, nacc], F32, tag="racc")
            c_eps = pp.tile([P, 1], F32, tag="c_eps")
            c_1eps = pp.tile([P, 1], F32, tag="c_1eps")
            nc.vector.memset(c_eps[:], EPS)
            nc.vector.memset(c_1eps[:], 1.0 + EPS)

            k = 0
            for s in range(SPC):
                for ch in range(NCHUNK):
                    sl = slice(ch * CHUNK, (ch + 1) * CHUNK)
                    ox = wp.tile([P, CHUNK], F32, tag="ox")
                    oy = wp.tile([P, CHUNK], F32, tag="oy")
                    ft = wp.tile([P, CHUNK], F32, tag="ft")
                    tt = wp.tile([P, CHUNK], I32, tag="tt")
                    nc.sync.dma_start(ox[:], off_v[s, 0][:, sl])
                    nc.sync.dma_start(oy[:], off_v[s, 1][:, sl])
                    nc.sync.dma_start(ft[:], f_v[s][:, sl])
                    nc.sync.dma_start(tt[:], t_v[s][:, sl])

                    # ix chain, in place on ox
                    nc.vector.scalar_tensor_tensor(
                        ox[:], ox[:], W / 2.0, base_x[:], Alu.mult, Alu.add
                    )
                    nc.vector.tensor_scalar(
                        ox[:], ox[:], 0.0, float(W - 1), Alu.max, Alu.min
                    )
                    nc.vector.tensor_scalar(
                        ox[:], ox[:], RC, RC, Alu.add, Alu.subtract
                    )
                    # iy chain; fold +s*H (table sample offset) into RNE subtract
                    nc.vector.scalar_tensor_tensor(
                        oy[:], oy[:], H / 2.0, base_ys[ch][:], Alu.mult, Alu.add
                    )
                    nc.vector.tensor_scalar(
                        oy[:], oy[:], 0.0, float(H - 1), Alu.max, Alu.min
                    )
                    nc.vector.tensor_scalar(
                        oy[:], oy[:], RC, RC - s * H, Alu.add, Alu.subtract
                    )
                    idx = wp.tile([P, CHUNK], I32, tag="idx")
                    nc.vector.scalar_tensor_tensor(
                        idx[:], oy[:], float(W), ox[:], Alu.mult, Alu.add
                    )

                    hs = wp.tile([P, CHUNK], I32, tag="hs")
                    gw = CHUNK // GSPLIT
                    for g in range(GSPLIT):
                        gs = slice(g * gw, (g + 1) * gw)
                        nc.gpsimd.indirect_dma_start(
                            out=hs[:, gs],
                            out_offset=None,
                            in_=tflat,
                            in_offset=IndirectOffsetOnAxis(ap=idx[:, gs], axis=0),
                        )

                    u = wp.tile([P, CHUNK], F32, tag="u")
                    v = wp.tile([P, CHUNK], F32, tag="v")
                    nc.scalar.activation(u[:], ft[:], Act.Ln, bias=c_eps[:], scale=1.0)
                    nc.scalar.activation(
                        v[:], ft[:], Act.Ln, bias=c_1eps[:], scale=-1.0,
                        accum_out=racc[:, 2 * k : 2 * k + 1],
                    )
                    nc.vector.tensor_tensor(u[:], u[:], v[:], Alu.subtract)  # w=u-v
                    nc.vector.tensor_tensor(ft[:], hs[:], tt[:], Alu.is_equal)
                    nc.vector.scalar_tensor_tensor(
                        ft[:], ft[:], 0.0, u[:], Alu.add, Alu.mult,
                        accum_out=racc[:, 2 * k + 1 : 2 * k + 2],
                    )
                    k += 1
            nc.sync.dma_start(out_d.ap(), racc[:])
    nc.finalize()
    return nc


_NC = None
LAST_RESULT = None


def kernel(offset, f, target):
    global _NC, LAST_RESULT
    if _NC is None:
        _NC = build()
    in_maps = []
    for c in range(NCORES):
        sl = slice(c * SPC, (c + 1) * SPC)
        in_maps.append(
            {
                "offset": np.ascontiguousarray(offset[sl], dtype=np.float32),
                "f": np.ascontiguousarray(
                    np.asarray(f)[sl].reshape(SPC, H, W), dtype=np.float32
                ),
                "target": np.ascontiguousarray(target[sl], dtype=np.int32),
            }
        )
    trace = bool(int(os.environ.get("CONF_TRACE", "0")))
    LAST_RESULT = run_bass_kernel_spmd(
        _NC, in_maps, core_ids=list(range(NCORES)), trace=trace
    )
    total = 0.0
    for r in LAST_RESULT.results:
        total += float(np.sum(r["out"].astype(np.float64)))
    return np.array(-total / (H * W), dtype=np.float32)
